# revision 2
# baseline (speedup 1.0000x reference)
"""Born-Wolf PSF kernel for Trainium2, 8 NeuronCores, data-parallel over batch.

Self-contained: hardcodes all geometry from the problem spec.
  input : params (16, 64, 2) float32
  output: psf    (16, 64, 25, 25, 25) float32

v3 design:
  - Every matmul-RHS row over the (pair, anchor) field is rank-1:
    row_k[(p,a)] = pairfac_k[p] * anchorfac_k[a].  Pair factors live in
    tP [128, 65] (kinv at col 32, wc at col 64 so their transposed rows
    land on legal partition bases), transposed ONCE via PE identity-
    transpose; each field chunk's RHS [16, 490] is one broadcast multiply.
  - J0 large branch drops the tiny a2 amplitude correction (<=0.5% local)
    making the amplitude rank-1: amp = a1[p,a] * S_j.  S_j is folded into
    the contraction's CT side, a1^2 into a single plane-level scale, and
    the poly (small-x) rows are pre-divided by S_j*a1 (stays rank-1).
    The Sin then writes tJ0 directly: no amp matmul, no multiply.
  - GPSIMD never touches PSUM (hw rule): each PSUM tile is copied once
    to SBUF (Act/DVE) and all Pool work is SBUF-side.
  - Output: the 25x25 plane has only U unique radii; device emits
    [128, 13, U]; host expands to 625 pixels (bitwise-equal gather) and
    reflect-mirrors z, both pure data movement done during unshard.
"""
import os
import numpy as np

# ---------------- problem geometry (hardcoded) ----------------
B, CH = 16, 64
NCORES = 8
NP = (B // NCORES) * CH          # 128 pairs per core
NA, NJ, NZH, NZ = 35, 101, 13, 25
F = NP * NA                      # 4480
NPOW = 8                         # poly terms m=0..7 (J0 err at x=4 ~4e-5)
XC = 4.0
PI = float(np.pi)
C0 = -0.1562499995e-1
S_AMP = float(np.sqrt(0.636619772))
QS = [1.0, -0.25, 0.015624999996, -0.00043402777473, 6.7816828549e-06,
      -6.781657507e-08, 4.7091319698e-10, -2.3995591574e-12]
MAGIC = 12582912.0               # 1.5 * 2**23
C2B = 0.1430488765e-3

# tLr (f32r matmul lhsT rows [16, LRCOLS]) column layout
PH = 0            # phase lhsT rows {0..3}
POL = 101         # poly lhsT rows {4..4+NPOW-1}
CL = 202          # c-stage lhsT row {0} = rho^2/2pi
ON = 303          # ones row {0}
A1R = 404         # a1^2 anchor row {0} = S_AMP^2 * Rinv  [1, 35]
LRCOLS = 439
# tLa (f32 broadcast tables [16, LACOLS])
AFT_ = 0          # anchor-factor rows [16, 35]
ZR = 35           # z row {0} [1, 13]
LACOLS = 48

FCH = 1664        # 13 * 128 c-stage cols per half
FP = [14] * 9 + [2]              # pairs per field chunk (pair-major)
FPO = [0, 14, 28, 42, 56, 70, 84, 98, 112, 126]
CP = [38, 38, 38, 14]            # pairs per c-stage chunk (even widths)
CPO = [0, 38, 76, 114]
C_AFTER = {0: [0], 1: [1], 4: [2], 6: [3]}
WAVE_AFTER = {2: [0], 3: [1], 4: [2], 5: [3], 6: [4], 7: [5], 8: [6],
              9: [7]}

_CACHE = {}


def _uidx():
    Y, X = np.meshgrid(np.arange(25), np.arange(25), indexing="ij")
    d2 = ((X - 12) ** 2 + (Y - 12) ** 2).astype(np.int64).ravel()
    uniq, uidx = np.unique(d2, return_inverse=True)
    return uniq, uidx


UNIQ, UIDX = _uidx()
U = len(UNIQ)
UP = U + (U % 2)                 # device width padded even for f32r matmuls
GU = UP


def _host_consts():
    if "consts" in _CACHE:
        return _CACHE["consts"]
    f32 = np.float32
    R = np.linspace(0, 34, NA) / 2.0
    RHO = np.linspace(0.0, 1.0, NJ)
    wt = np.full(NJ, 0.01)
    wt[0] *= 0.5
    wt[-1] *= 0.5
    rw = RHO * wt
    rinv = np.zeros(NJ)
    rinv[1:] = 1.0 / RHO[1:]
    Rinv = np.zeros(NA)
    Rinv[1:] = 1.0 / R[1:]

    tLr = np.zeros((16, LRCOLS))
    tLr[0, PH:PH + NJ] = RHO / (2 * PI)
    tLr[1, PH:PH + NJ] = rinv / (2 * PI)
    tLr[2, PH:PH + NJ] = rinv ** 3 / (2 * PI)
    tLr[3, PH:PH + NJ] = 0.125
    # poly rows pre-divided by S_j = sqrt(rinv)*rw:  rw/S = sqrt(rho)
    for m in range(NPOW):
        tLr[4 + m, POL:POL + NJ] = (QS[m] * (400.0 * RHO ** 2) ** m
                                    * np.sqrt(RHO))
    tLr[0, CL:CL + NJ] = RHO ** 2 / (2 * PI)
    tLr[0, ON:ON + NJ] = 1.0
    a1sq_row = S_AMP * S_AMP * Rinv
    a1sq_row[0] = 1.0   # anchor 0: plane = (sum rw*ct)^2 via m=0 poly slot
    tLr[0, A1R:A1R + NA] = a1sq_row

    tLa = np.zeros((16, LACOLS))
    aft = np.zeros((16, NA))
    aft[0] = R
    aft[1] = 8 * C0 * Rinv
    aft[2] = 512 * C2B * Rinv ** 3
    aft[3] = 1.0
    # poly anchor factors: (R/20)^{2m} * sqrt(R)/S_AMP   ([0]=0)
    for m in range(NPOW):
        aft[4 + m] = (R / 20.0) ** (2 * m) * np.sqrt(R) / S_AMP
        aft[4 + m, 0] = 0.0
    aft[4, 0] = 1.0
    tLa[:, AFT_:AFT_ + NA] = aft
    tLa[0, ZR:ZR + NZH] = np.arange(NZH)

    # threshold table + S column: [101, 36]
    rrm = np.outer(RHO, R)
    with np.errstate(divide="ignore"):
        thr = np.where(rrm > 0, XC / np.maximum(rrm, 1e-12), 1e9)
    thr = np.minimum(thr, 1e9)
    tTh = np.zeros((NJ, NA + 1))
    tTh[:, 0:NA] = thr
    tTh[:, NA] = np.sqrt(rinv) * rw          # S_j

    rpu = np.sqrt(UNIQ.astype(np.float64))
    i1 = np.floor(rpu * 2).astype(np.int64)
    i2 = i1 + 1
    d1 = (rpu - R[i1]) * 2.0
    d2_ = 1.0 - d1
    Gp = np.zeros((NA, UP))
    for u in range(U):
        Gp[i2[u], u] += d1[u]
        Gp[i1[u], u] += d2_[u]
    counts = np.bincount(UIDX, minlength=UP).astype(np.float64)
    gcol = Gp @ counts
    gcolN = np.zeros((NA, 6))
    gcolN[:, 0] = gcol            # z = 0 (weight 1)
    gcolN[:, 3] = gcol * 2.0      # z odd
    gcolN[:, 4] = gcol * 2.0      # z even > 0
    tG = np.concatenate([Gp, gcolN], axis=1)

    consts = {
        "tLr": tLr.astype(f32),
        "tLa": tLa.astype(f32),
        "tG": tG.astype(f32),
        "tThr": tTh.astype(f32),
        "ident": np.eye(128, dtype=f32),
    }
    for k, v in consts.items():
        assert np.isfinite(v).all(), k
    _CACHE["consts"] = consts
    return consts


def _ensure_paths():
    import sys
    for p in ("/opt/trn_rl_repo", "/root/.axon_site/_ro/trn_rl_repo"):
        if os.path.isdir(p) and p not in sys.path:
            sys.path.append(p)


def _build_nc():
    if "nc" in _CACHE:
        return _CACHE["nc"]
    _ensure_paths()
    from contextlib import ExitStack
    import concourse.bass as bass
    import concourse.bacc as bacc
    import concourse.tile as tile
    from concourse import mybir

    f32 = mybir.dt.float32
    f32r = mybir.dt.float32r
    bf16 = mybir.dt.bfloat16
    i32 = mybir.dt.int32
    AF = mybir.ActivationFunctionType
    OP = mybir.AluOpType

    nc = bacc.Bacc()

    d_par = nc.declare_dram_parameter("params", [NP, 2], f32, isOutput=False)
    d_tLr = nc.declare_dram_parameter("tLr", [16, LRCOLS], f32r,
                                      isOutput=False)
    d_tLa = nc.declare_dram_parameter("tLa", [16, LACOLS], f32,
                                      isOutput=False)
    d_tG = nc.declare_dram_parameter("tG", [NA, UP + 6], f32r,
                                     isOutput=False)
    d_thr = nc.declare_dram_parameter("tThr", [NJ, NA + 1], f32,
                                      isOutput=False)
    d_id = nc.declare_dram_parameter("ident", [128, 128], f32, isOutput=False)
    d_out = nc.declare_dram_parameter("out", [NP, NZH * UP], f32,
                                     isOutput=True)

    with tile.TileContext(nc) as tc, ExitStack() as ctx:
        p1 = ctx.enter_context(tc.tile_pool(name="p1", bufs=1))
        pcos = ctx.enter_context(tc.tile_pool(name="pcos", bufs=8))

        # ---- const + input loads ----
        t_par = p1.tile([NP, 2], f32, tag="par")
        tLt = p1.tile([16, LRCOLS], f32r, tag="tLr")
        tLa = p1.tile([16, LACOLS], f32, tag="tLa")
        tGt = p1.tile([NA, UP + 6], f32r, tag="tG")
        tTh = p1.tile([NJ, NA + 1], f32, tag="thr")
        tI = p1.tile([128, 128], f32, tag="ident")
        nc.sync.dma_start(out=t_par[:], in_=d_par[:])
        nc.sync.dma_start(out=tI[:], in_=d_id[:])
        nc.scalar.dma_start(out=tLt[:], in_=d_tLr[:])
        nc.scalar.dma_start(out=tLa[:], in_=d_tLa[:])
        nc.gpsimd.dma_start(out=tTh[:], in_=d_thr[:])
        nc.sync.dma_start(out=tGt[:], in_=d_tG[:])

        # preload the trig_and_small activation table early
        t_scr = p1.tile([1, 1], f32, tag="scr")
        nc.scalar.activation(t_scr[:], tLa[0:1, 0:1], AF.Sin, scale=1.0)

        # ---- pair factors tP [128, 65] ----
        # slots: 0 kn, 1 kinv, 2 ki3, 3 ones, 4 knh=sqrt(kn),
        #        5..4+NPOW-1: kn^{2m}*knh (m=1..7), 32 kinv, 64 wc
        tP = p1.tile([NP, 65], f32, tag="tP")
        t_abs = p1.tile([NP, 2], f32, tag="abs")
        nc.vector.scalar_tensor_tensor(t_abs[:], t_par[:], -1.0, t_par[:],
                                       OP.mult, OP.max)
        lam = t_abs[:, 0:1]
        enn = t_abs[:, 1:2]
        t_rl = p1.tile([NP, 1], f32, tag="rl")
        t_rn = p1.tile([NP, 1], f32, tag="rn")
        nc.vector.reciprocal(t_rl[:], lam)
        nc.vector.reciprocal(t_rn[:], enn)
        kn = tP[:, 0:1]
        nc.vector.scalar_tensor_tensor(kn, t_rl[:], 2.0 * PI, enn,
                                       OP.mult, OP.mult)
        kinv = tP[:, 1:2]
        nc.vector.scalar_tensor_tensor(kinv, lam, 1.0 / (2.0 * PI), t_rn[:],
                                       OP.mult, OP.mult)
        t_ki2 = p1.tile([NP, 1], f32, tag="ki2")
        nc.vector.tensor_tensor(t_ki2[:], kinv, kinv, OP.mult)
        nc.vector.tensor_tensor(tP[:, 2:3], t_ki2[:], kinv, OP.mult)
        nc.vector.memset(tP[:, 3:4], 1.0)
        nc.vector.memset(tP[:, 12:32], 0.0)
        nc.vector.memset(tP[:, 33:64], 0.0)
        # rsqrt(kn): quake seed + 2 Newton steps
        t_sh = p1.tile([NP, 1], i32, tag="sh")
        nc.vector.tensor_scalar(t_sh[:], kn.bitcast(i32), 1, None,
                                OP.arith_shift_right)
        nc.vector.tensor_scalar(t_sh[:], t_sh[:], -1, None, OP.bitwise_xor)
        t_y = p1.tile([NP, 1], f32, tag="y")
        nc.vector.tensor_scalar(t_y[:].bitcast(i32), t_sh[:],
                                0x5F3759DF + 1, None, OP.add)
        t_xh = p1.tile([NP, 1], f32, tag="xh")
        nc.vector.tensor_scalar(t_xh[:], kn, -0.5, None, OP.mult)
        t_t1 = p1.tile([NP, 1], f32, tag="t1")
        t_t2 = p1.tile([NP, 1], f32, tag="t2")
        t_rsk = p1.tile([NP, 1], f32, tag="rsk")
        for it in range(2):
            nc.vector.tensor_tensor(t_t1[:], t_y[:], t_y[:], OP.mult)
            nc.vector.tensor_tensor(t_t2[:], t_t1[:], t_xh[:], OP.mult)
            nc.vector.tensor_scalar(t_t2[:], t_t2[:], 1.5, None, OP.add)
            dst = t_rsk[:] if it == 1 else t_y[:]
            nc.vector.tensor_tensor(dst, t_y[:], t_t2[:], OP.mult)
        # knh = sqrt(kn) = kn * rsqrt(kn)
        knh = tP[:, 4:5]
        nc.vector.tensor_tensor(knh, kn, t_rsk[:], OP.mult)
        kn2 = t_t1[:]
        nc.vector.tensor_tensor(kn2, kn, kn, OP.mult)
        kn4 = t_t2[:]
        nc.vector.tensor_tensor(kn4, kn2, kn2, OP.mult)
        # pow slots 5..11: kn^{2m}*knh for m=1..7
        nc.vector.tensor_tensor(tP[:, 5:6], kn2, knh, OP.mult)
        nc.vector.tensor_tensor(tP[:, 6:7], tP[:, 5:6], kn2, OP.mult)
        nc.vector.tensor_tensor(tP[:, 7:9], tP[:, 5:7],
                                kn4.broadcast_to((NP, 2)), OP.mult)
        kn8 = p1.tile([NP, 1], f32, tag="kn8")
        nc.vector.tensor_tensor(kn8[:], kn4, kn4, OP.mult)
        nc.vector.tensor_tensor(tP[:, 9:12], tP[:, 5:8],
                                kn8[:].broadcast_to((NP, 3)), OP.mult)
        nc.vector.tensor_tensor(tP[:, 32:33], kinv, tP[:, 3:4], OP.mult)
        nc.vector.scalar_tensor_tensor(tP[:, 64:65], kn, 0.5, enn,
                                       OP.mult, OP.mult)

        tPT = p1.tile([16, 128], f32r, tag="tPT")
        tKi = p1.tile([1, 128], f32r, tag="tKi")
        tWc = p1.tile([1, 128], f32, tag="tWc")
        tKn = p1.tile([NJ, 128], f32, tag="tKn")
        tW0 = p1.tile([1, FCH], f32r, tag="tW0")
        tJ0 = p1.tile([NJ, F], bf16, tag="J0")
        tCT = p1.tile([NJ, 2 * FCH], bf16, tag="CT")
        tCS = p1.tile([NJ, 2 * FCH], bf16, tag="CS")
        tMask = p1.tile([NJ, F], i32, tag="mask")
        tBgF = p1.tile([16, F], f32r, tag="bgF")
        tUV = p1.tile([NA, NP * 26], f32, tag="uv")
        tPL = p1.tile([NA, NZH * NP], f32r, tag="pl")
        tA1 = p1.tile([NA, NP], f32, tag="a1")
        tOut = p1.tile([NP, NZH * UP], f32, tag="out")

        c3 = tCS[:].rearrange("j (h p z) -> j p h z", h=2, z=NZH)
        uv3 = tUV[:].rearrange("q (pp c) -> q pp c", c=26)
        pl3 = tPL[:].rearrange("q (zz pp) -> q pp zz", pp=128)

        with ExitStack() as fld:
            p_t0 = fld.enter_context(
                tc.tile_pool(name="pt0", bufs=2, space="PSUM"))
            p_pj = fld.enter_context(
                tc.tile_pool(name="ppj", bufs=2, space="PSUM"))
            p_w = fld.enter_context(
                tc.tile_pool(name="pw", bufs=2, space="PSUM"))
            ppr = fld.enter_context(
                tc.tile_pool(name="ppr", bufs=2, space="PSUM"))

            # ---- one transpose: [128,65] -> [65,128] ----
            psP = p_t0.tile([65, 128], f32, tag="t0", name="psP")
            nc.tensor.transpose(psP[:], tP[:], tI[:])
            nc.vector.tensor_scalar(tPT[:], psP[0:16, :], 1.0, None, OP.mult)
            nc.vector.tensor_scalar(tKi[:], psP[32:33, :], 1.0, None, OP.mult)
            nc.vector.tensor_scalar(tWc[:], psP[64:65, :], 1.0, None, OP.mult)
            # kn replicated over 101 partitions (mask compare)
            psKn = p_t0.tile([NJ, 128], f32, tag="t0", name="psKn")
            nc.tensor.matmul(psKn[:], tLt[0:1, ON:ON + NJ], tPT[0:1, :],
                             start=True, stop=True)
            nc.vector.tensor_scalar(tKn[:], psKn[:], 1.0, None, OP.mult)
            # a1^2 = (S_AMP^2 Rinv)[a] * kinv[p]  (rank-1, PE outer product)
            psA1 = p_pj.tile([NA, 128], f32, tag="pj", name="psA1")
            nc.tensor.matmul(psA1[:], tLt[0:1, A1R:A1R + NA], tKi[0:1, :],
                             start=True, stop=True)
            nc.vector.tensor_scalar(tA1[:], psA1[:], 1.0, None, OP.mult)
            # c-stage w row: w0[(p,z)] = wc[p] * z
            w03 = tW0[:].rearrange("q (p z) -> q p z", z=NZH)
            nc.gpsimd.tensor_tensor(
                w03, tWc[:, :, None].broadcast_to((1, 128, NZH)),
                tLa[0:1, ZR:ZR + NZH][:, None, :].broadcast_to((1, 128, NZH)),
                OP.mult)

            def prep(c):
                if c >= 10:
                    return
                np_ = FP[c]
                p0 = FPO[c]
                sl = slice(p0 * NA, (p0 + np_) * NA)
                nc.gpsimd.tensor_tensor(
                    tBgF[:, sl].rearrange("k (p a) -> k p a", a=NA),
                    tPT[:, p0:p0 + np_][:, :, None]
                        .broadcast_to((16, np_, NA)),
                    tLa[:, AFT_:AFT_ + NA][:, None, :]
                        .broadcast_to((16, np_, NA)),
                    OP.mult)
                mke = nc.vector
                mke.tensor_tensor(
                    tMask[:, sl].rearrange("j (p a) -> j p a", a=NA),
                    tKn[:, p0:p0 + np_][:, :, None]
                        .broadcast_to((NJ, np_, NA)).bitcast(i32),
                    tTh[:, 0:NA][:, None, :].broadcast_to((NJ, np_, NA))
                        .bitcast(i32),
                    OP.is_lt)

            def field_chunk(c):
                np_ = FP[c]
                p0 = FPO[c]
                w = np_ * NA
                sl = slice(p0 * NA, p0 * NA + w)
                rhs = tBgF[0:16, sl]
                ps_t0 = p_t0.tile([NJ, 512], f32, tag="t0", name="ps_t0")
                nc.tensor.matmul(ps_t0[:, 0:w], tLt[0:16, PH:PH + NJ], rhs,
                                 start=True, stop=True)
                prep(c + 1)
                # PSUM -> SBUF copy (Act even / DVE odd), then SBUF-side ops
                tT0 = pcos.tile([NJ, 512], f32, tag="t0s", name="tT0")
                if c % 2 == 0:
                    nc.scalar.activation(tT0[:, 0:w], ps_t0[:, 0:w], AF.Copy)
                else:
                    nc.vector.tensor_scalar(tT0[:, 0:w], ps_t0[:, 0:w], 1.0,
                                            None, OP.mult)
                tRR = pcos.tile([NJ, 512], f32, tag="rr", name="tRR")
                nc.vector.tensor_scalar(tRR[:, 0:w], tT0[:, 0:w], MAGIC,
                                        MAGIC, OP.add, OP.subtract)
                tU = pcos.tile([NJ, 512], f32, tag="u", name="tU")
                nc.gpsimd.tensor_tensor(tU[:, 0:w], tT0[:, 0:w],
                                        tRR[:, 0:w], OP.subtract)
                nc.scalar.activation(tJ0[:, sl], tU[:, 0:w], AF.Sin,
                                     scale=2.0 * PI)
                ps_pj = p_pj.tile([NJ, 512], f32, tag="pj", name="ps_pj")
                nc.tensor.matmul(ps_pj[:, 0:w], tLt[0:16, POL:POL + NJ], rhs,
                                 start=True, stop=True)
                nc.vector.copy_predicated(tJ0[:, sl], tMask[:, sl],
                                          ps_pj[:, 0:w])

            def c_chunk(k):
                np_ = CP[k]
                o = CPO[k] * NZH
                w = np_ * NZH
                ps_w = p_w.tile([NJ, 512], f32, tag="w", name="ps_w")
                nc.tensor.matmul(ps_w[:, 0:w], tLt[0:1, CL:CL + NJ],
                                 tW0[0:1, o:o + w], start=True, stop=True)
                tWs = pcos.tile([NJ, 512], f32, tag="ws", name="tWs")
                if k % 2 == 0:
                    nc.vector.tensor_scalar(tWs[:, 0:w], ps_w[:, 0:w], 1.0,
                                            None, OP.mult)
                else:
                    nc.scalar.activation(tWs[:, 0:w], ps_w[:, 0:w], AF.Copy)
                tRs = pcos.tile([NJ, 512], f32, tag="rr", name="tRs")
                nc.vector.tensor_scalar(tRs[:, 0:w], tWs[:, 0:w], MAGIC,
                                        MAGIC, OP.add, OP.subtract)
                tUs = pcos.tile([NJ, 512], f32, tag="u", name="tUs")
                nc.gpsimd.tensor_tensor(tUs[:, 0:w], tWs[:, 0:w],
                                        tRs[:, 0:w], OP.subtract)
                nc.scalar.activation(tCT[:, o:o + w], tUs[:, 0:w], AF.Sin,
                                     scale=2.0 * PI)
                nc.vector.tensor_scalar(tCS[:, o:o + w], tCT[:, o:o + w],
                                        tTh[:, NA:NA + 1], None, OP.mult)
                tXc = pcos.tile([NJ, 512], f32, tag="xc", name="tXc")
                nc.gpsimd.tensor_scalar(tXc[:, 0:w], tWs[:, 0:w], 0.25,
                                        None, OP.add)
                tRc = pcos.tile([NJ, 512], f32, tag="rr", name="tRc")
                nc.vector.tensor_scalar(tRc[:, 0:w], tXc[:, 0:w], MAGIC,
                                        MAGIC, OP.add, OP.subtract)
                tUc = pcos.tile([NJ, 512], f32, tag="u", name="tUc")
                nc.gpsimd.tensor_tensor(tUc[:, 0:w], tXc[:, 0:w],
                                        tRc[:, 0:w], OP.subtract)
                nc.scalar.activation(tCT[:, FCH + o:FCH + o + w],
                                     tUc[:, 0:w], AF.Sin, scale=2.0 * PI)
                nc.vector.tensor_scalar(tCS[:, FCH + o:FCH + o + w],
                                        tCT[:, FCH + o:FCH + o + w],
                                        tTh[:, NA:NA + 1], None, OP.mult)

            def wave(wv):
                tPRw = ppr.tile([NA, 512], f32, tag="PR", name="tPRw")
                for j in range(16):
                    p = wv * 16 + j
                    nc.tensor.matmul(tPRw[:, j * 32:j * 32 + 26],
                                     tJ0[:, p * NA:(p + 1) * NA],
                                     c3[:, p], start=True, stop=True)
                pr4 = tPRw[:].rearrange("q (n s) -> q n s", s=32)
                sl = slice(wv * 16, (wv + 1) * 16)
                nc.scalar.activation(uv3[:, sl, :], pr4[:, :, 0:26],
                                     AF.Square)
                nc.gpsimd.tensor_tensor(pl3[:, sl, :], uv3[:, sl, 0:NZH],
                                        uv3[:, sl, NZH:26], OP.add)
                nc.gpsimd.tensor_tensor(
                    pl3[:, sl, :], pl3[:, sl, :],
                    tA1[:, sl][:, :, None].broadcast_to((NA, 16, NZH)),
                    OP.mult)

            prep(0)
            for c in range(10):
                field_chunk(c)
                for k in C_AFTER.get(c, []):
                    c_chunk(k)
                for wv in WAVE_AFTER.get(c, []):
                    wave(wv)

        # ---- normalization + unique-radius expansion + output ----
        pnn = ctx.enter_context(tc.tile_pool(name="pnn", bufs=1, space="PSUM"))
        pgo = ctx.enter_context(tc.tile_pool(name="pgo", bufs=4, space="PSUM"))
        ps_n = pnn.tile([NP, 2], f32, tag="N")
        for zz in range(NZH):
            off = GU if zz == 0 else (GU + 2 if zz % 2 else GU + 4)
            nc.tensor.matmul(ps_n[:], tPL[:, zz * NP:(zz + 1) * NP],
                             tGt[:, off:off + 2],
                             start=(zz == 0), stop=(zz == NZH - 1))
        t_ns = p1.tile([NP, 2], f32, tag="ns")
        nc.scalar.activation(t_ns[:], ps_n[:], AF.Copy)
        t_nr = p1.tile([NP, 1], f32, tag="nr")
        nc.vector.tensor_tensor(t_nr[:], t_ns[:, 0:1], t_ns[:, 1:2], OP.add)
        t_rcp = p1.tile([NP, 1], f32, tag="rcp")
        nc.vector.reciprocal(t_rcp[:], t_nr[:])
        for g in range(7):
            z0 = 2 * g
            nz = min(2, NZH - z0)
            tOC = pgo.tile([NP, 2 * UP], f32, tag="OC", name="tOC")
            for k in range(nz):
                nc.tensor.matmul(tOC[:, k * UP:(k + 1) * UP],
                                 tPL[:, (z0 + k) * NP:(z0 + k + 1) * NP],
                                 tGt[:, 0:UP], start=True, stop=True)
            osl = tOut[:, z0 * UP:(z0 + nz) * UP]
            if g % 2 == 0:
                nc.scalar.activation(osl, tOC[:, 0:nz * UP], AF.Copy,
                                     scale=t_rcp[:, 0:1])
            else:
                nc.vector.tensor_scalar(osl, tOC[:, 0:nz * UP],
                                        t_rcp[:, 0:1], None, OP.mult)
            if g >= 2:
                a = z0 * UP if g > 2 else 0
                b = (z0 + nz) * UP
                deng = (nc.sync, nc.scalar, nc.sync, nc.scalar,
                        nc.sync)[g - 2]
                deng.dma_start(out=d_out[:, a:b], in_=tOut[:, a:b])

    nc.finalize()
    _CACHE["nc"] = nc
    return nc


def _expand(core_out):
    """[NP, 13*U] device output -> [NP, 25, 25, 25] full psf (pure gather)."""
    r = core_out.reshape(NP, NZH, UP)
    p625 = r[:, :, UIDX]
    full = np.concatenate([p625[:, 12:0:-1, :], p625], axis=1)
    return full.reshape(NP, NZ, 25, 25)


def kernel(params):
    _ensure_paths()
    from concourse.bass_utils import run_bass_kernel_spmd

    params = np.asarray(params, dtype=np.float32)
    assert params.shape == (B, CH, 2)
    consts = _host_consts()
    nc = _build_nc()

    per = B // NCORES
    in_maps = []
    for i in range(NCORES):
        m = {"params": params[i * per:(i + 1) * per].reshape(NP, 2).copy()}
        m.update(consts)
        in_maps.append(m)

    res = run_bass_kernel_spmd(nc, in_maps, list(range(NCORES)))

    out = np.empty((B, CH, NZ, 25, 25), np.float32)
    for i in range(NCORES):
        full = _expand(np.asarray(res.results[i]["out"]))
        out[i * per:(i + 1) * per] = full.reshape(per, CH, NZ, 25, 25)
    return out


# revision 3
# speedup vs baseline: 1.0312x; 1.0312x over previous
"""Born-Wolf PSF kernel for Trainium2, 8 NeuronCores, data-parallel over batch.

Self-contained: hardcodes all geometry from the problem spec.
  input : params (16, 64, 2) float32
  output: psf    (16, 64, 25, 25, 25) float32

v3 design:
  - Every matmul-RHS row over the (pair, anchor) field is rank-1:
    row_k[(p,a)] = pairfac_k[p] * anchorfac_k[a].  Pair factors live in
    tP [128, 65] (kinv at col 32, wc at col 64 so their transposed rows
    land on legal partition bases), transposed ONCE via PE identity-
    transpose; each field chunk's RHS [16, 490] is one broadcast multiply.
  - J0 large branch drops the tiny a2 amplitude correction (<=0.5% local)
    making the amplitude rank-1: amp = a1[p,a] * S_j.  S_j is folded into
    the contraction's CT side, a1^2 into a single plane-level scale, and
    the poly (small-x) rows are pre-divided by S_j*a1 (stays rank-1).
    The Sin then writes tJ0 directly: no amp matmul, no multiply.
  - GPSIMD never touches PSUM (hw rule): each PSUM tile is copied once
    to SBUF (Act/DVE) and all Pool work is SBUF-side.
  - Output: the 25x25 plane has only U unique radii; device emits
    [128, 13, U]; host expands to 625 pixels (bitwise-equal gather) and
    reflect-mirrors z, both pure data movement done during unshard.
"""
import os
import numpy as np

# ---------------- problem geometry (hardcoded) ----------------
B, CH = 16, 64
NCORES = 8
NP = (B // NCORES) * CH          # 128 pairs per core
NA, NJ, NZH, NZ = 35, 101, 13, 25
F = NP * NA                      # 4480
NPOW = 8                         # poly terms m=0..7 (J0 err at x=4 ~4e-5)
XC = 4.0
PI = float(np.pi)
C0 = -0.1562499995e-1
S_AMP = float(np.sqrt(0.636619772))
QS = [1.0, -0.25, 0.015624999996, -0.00043402777473, 6.7816828549e-06,
      -6.781657507e-08, 4.7091319698e-10, -2.3995591574e-12]
MAGIC = 12582912.0               # 1.5 * 2**23
C2B = 0.1430488765e-3

# tLr (f32r matmul lhsT rows [16, LRCOLS]) column layout
PH = 0            # phase lhsT rows {0..3}
POL = 101         # poly lhsT rows {4..4+NPOW-1}
CL = 202          # c-stage lhsT row {0} = rho^2/2pi
ON = 303          # ones row {0}
A1R = 404         # a1^2 anchor row {0} = S_AMP^2 * Rinv  [1, 35]
LRCOLS = 439
# tLa (f32 broadcast tables [16, LACOLS])
AFT_ = 0          # anchor-factor rows [16, 35]
ZR = 35           # z row {0} [1, 13]
LACOLS = 48

FCH = 1664        # 13 * 128 c-stage cols per half
FP = [14] * 9 + [2]              # pairs per field chunk (pair-major)
FPO = [0, 14, 28, 42, 56, 70, 84, 98, 112, 126]
CP = [38, 38, 38, 14]            # pairs per c-stage chunk (even widths)
CPO = [0, 38, 76, 114]
C_AFTER = {0: [0], 1: [1], 3: [2], 5: [3]}
WAVE_AFTER = {1: [0], 2: [1], 3: [2], 4: [3], 5: [4], 6: [5], 7: [6],
              9: [7]}

_CACHE = {}


def _uidx():
    Y, X = np.meshgrid(np.arange(25), np.arange(25), indexing="ij")
    d2 = ((X - 12) ** 2 + (Y - 12) ** 2).astype(np.int64).ravel()
    uniq, uidx = np.unique(d2, return_inverse=True)
    return uniq, uidx


UNIQ, UIDX = _uidx()
U = len(UNIQ)
UP = U + (U % 2)                 # device width padded even for f32r matmuls
GU = UP


def _host_consts():
    if "consts" in _CACHE:
        return _CACHE["consts"]
    f32 = np.float32
    R = np.linspace(0, 34, NA) / 2.0
    RHO = np.linspace(0.0, 1.0, NJ)
    wt = np.full(NJ, 0.01)
    wt[0] *= 0.5
    wt[-1] *= 0.5
    rw = RHO * wt
    rinv = np.zeros(NJ)
    rinv[1:] = 1.0 / RHO[1:]
    Rinv = np.zeros(NA)
    Rinv[1:] = 1.0 / R[1:]

    tLr = np.zeros((16, LRCOLS))
    tLr[0, PH:PH + NJ] = RHO / (2 * PI)
    tLr[1, PH:PH + NJ] = rinv / (2 * PI)
    tLr[2, PH:PH + NJ] = rinv ** 3 / (2 * PI)
    tLr[3, PH:PH + NJ] = 0.125
    # poly rows pre-divided by S_j = sqrt(rinv)*rw:  rw/S = sqrt(rho)
    for m in range(NPOW):
        tLr[4 + m, POL:POL + NJ] = (QS[m] * (400.0 * RHO ** 2) ** m
                                    * np.sqrt(RHO))
    tLr[0, CL:CL + NJ] = RHO ** 2 / (2 * PI)
    tLr[0, ON:ON + NJ] = 1.0
    a1sq_row = S_AMP * S_AMP * Rinv
    a1sq_row[0] = 1.0   # anchor 0: plane = (sum rw*ct)^2 via m=0 poly slot
    tLr[0, A1R:A1R + NA] = a1sq_row

    tLa = np.zeros((16, LACOLS))
    aft = np.zeros((16, NA))
    aft[0] = R
    aft[1] = 8 * C0 * Rinv
    aft[2] = 512 * C2B * Rinv ** 3
    aft[3] = 1.0
    # poly anchor factors: (R/20)^{2m} * sqrt(R)/S_AMP   ([0]=0)
    for m in range(NPOW):
        aft[4 + m] = (R / 20.0) ** (2 * m) * np.sqrt(R) / S_AMP
        aft[4 + m, 0] = 0.0
    aft[4, 0] = 1.0
    tLa[:, AFT_:AFT_ + NA] = aft
    tLa[0, ZR:ZR + NZH] = np.arange(NZH)

    # threshold table + S column: [101, 36]
    rrm = np.outer(RHO, R)
    with np.errstate(divide="ignore"):
        thr = np.where(rrm > 0, XC / np.maximum(rrm, 1e-12), 1e9)
    thr = np.minimum(thr, 1e9)
    tTh = np.zeros((NJ, NA + 1))
    tTh[:, 0:NA] = thr
    tTh[:, NA] = np.sqrt(rinv) * rw          # S_j

    rpu = np.sqrt(UNIQ.astype(np.float64))
    i1 = np.floor(rpu * 2).astype(np.int64)
    i2 = i1 + 1
    d1 = (rpu - R[i1]) * 2.0
    d2_ = 1.0 - d1
    Gp = np.zeros((NA, UP))
    for u in range(U):
        Gp[i2[u], u] += d1[u]
        Gp[i1[u], u] += d2_[u]
    counts = np.bincount(UIDX, minlength=UP).astype(np.float64)
    gcol = Gp @ counts
    gcolN = np.zeros((NA, 4))
    gcolN[:, 0] = gcol            # z = 0 (weight 1)
    gcolN[:, 1] = gcol
    gcolN[:, 2] = gcol * 2.0      # z > 0
    gcolN[:, 3] = gcol * 2.0
    tG = np.concatenate([Gp, gcolN], axis=1)

    consts = {
        "tLr": tLr.astype(f32),
        "tLa": tLa.astype(f32),
        "tG": tG.astype(f32),
        "tThr": tTh.astype(f32),
        "ident": np.eye(128, dtype=f32),
    }
    for k, v in consts.items():
        assert np.isfinite(v).all(), k
    _CACHE["consts"] = consts
    return consts


def _ensure_paths():
    import sys
    for p in ("/opt/trn_rl_repo", "/root/.axon_site/_ro/trn_rl_repo"):
        if os.path.isdir(p) and p not in sys.path:
            sys.path.append(p)


def _build_nc():
    if "nc" in _CACHE:
        return _CACHE["nc"]
    _ensure_paths()
    from contextlib import ExitStack
    import concourse.bass as bass
    import concourse.bacc as bacc
    import concourse.tile as tile
    from concourse import mybir

    f32 = mybir.dt.float32
    f32r = mybir.dt.float32r
    bf16 = mybir.dt.bfloat16
    i32 = mybir.dt.int32
    AF = mybir.ActivationFunctionType
    OP = mybir.AluOpType

    nc = bacc.Bacc()

    d_par = nc.declare_dram_parameter("params", [NP, 2], f32, isOutput=False)
    d_tLr = nc.declare_dram_parameter("tLr", [16, LRCOLS], f32r,
                                      isOutput=False)
    d_tLa = nc.declare_dram_parameter("tLa", [16, LACOLS], f32,
                                      isOutput=False)
    d_tG = nc.declare_dram_parameter("tG", [NA, UP + 4], f32r,
                                     isOutput=False)
    d_thr = nc.declare_dram_parameter("tThr", [NJ, NA + 1], f32,
                                      isOutput=False)
    d_id = nc.declare_dram_parameter("ident", [128, 128], f32, isOutput=False)
    d_out = nc.declare_dram_parameter("out", [NP, NZH * UP], f32,
                                     isOutput=True)

    with tile.TileContext(nc) as tc, ExitStack() as ctx:
        p1 = ctx.enter_context(tc.tile_pool(name="p1", bufs=1))
        pcos = ctx.enter_context(tc.tile_pool(name="pcos", bufs=10))

        # ---- const + input loads ----
        t_par = p1.tile([NP, 2], f32, tag="par")
        tLt = p1.tile([16, LRCOLS], f32r, tag="tLr")
        tLa = p1.tile([16, LACOLS], f32, tag="tLa")
        tGt = p1.tile([NA, UP + 4], f32r, tag="tG")
        tTh = p1.tile([NJ, NA + 1], f32, tag="thr")
        tI = p1.tile([128, 128], f32, tag="ident")
        nc.sync.dma_start(out=t_par[:], in_=d_par[:])
        nc.sync.dma_start(out=tI[:], in_=d_id[:])
        nc.sync.dma_start(out=tLt[:], in_=d_tLr[:])
        nc.gpsimd.dma_start(out=tLa[:], in_=d_tLa[:])
        nc.gpsimd.dma_start(out=tTh[:], in_=d_thr[:])
        nc.sync.dma_start(out=tGt[:], in_=d_tG[:])

        # preload the trig_and_small activation table early
        t_scr = p1.tile([1, 1], f32, tag="scr")
        nc.scalar.activation(t_scr[:], tLa[0:1, 0:1], AF.Sin, scale=1.0)

        # ---- pair factors tP [128, 65] ----
        # slots: 0 kn, 1 kinv, 2 ki3, 3 ones, 4 knh=sqrt(kn),
        #        5..4+NPOW-1: kn^{2m}*knh (m=1..7), 32 kinv, 64 wc
        tP = p1.tile([NP, 65], f32, tag="tP")
        t_abs = p1.tile([NP, 2], f32, tag="abs")
        nc.vector.scalar_tensor_tensor(t_abs[:], t_par[:], -1.0, t_par[:],
                                       OP.mult, OP.max)
        lam = t_abs[:, 0:1]
        enn = t_abs[:, 1:2]
        t_rl = p1.tile([NP, 1], f32, tag="rl")
        t_rn = p1.tile([NP, 1], f32, tag="rn")
        nc.vector.reciprocal(t_rl[:], lam)
        nc.vector.reciprocal(t_rn[:], enn)
        kn = tP[:, 0:1]
        nc.vector.scalar_tensor_tensor(kn, t_rl[:], 2.0 * PI, enn,
                                       OP.mult, OP.mult)
        kinv = tP[:, 1:2]
        nc.vector.scalar_tensor_tensor(kinv, lam, 1.0 / (2.0 * PI), t_rn[:],
                                       OP.mult, OP.mult)
        t_ki2 = p1.tile([NP, 1], f32, tag="ki2")
        nc.vector.tensor_tensor(t_ki2[:], kinv, kinv, OP.mult)
        nc.vector.tensor_tensor(tP[:, 2:3], t_ki2[:], kinv, OP.mult)
        nc.vector.memset(tP[:, 3:4], 1.0)
        nc.vector.memset(tP[:, 12:32], 0.0)
        nc.vector.memset(tP[:, 33:64], 0.0)
        # rsqrt(kn): quake seed + 2 Newton steps
        t_sh = p1.tile([NP, 1], i32, tag="sh")
        nc.vector.tensor_scalar(t_sh[:], kn.bitcast(i32), 1, None,
                                OP.arith_shift_right)
        nc.vector.tensor_scalar(t_sh[:], t_sh[:], -1, None, OP.bitwise_xor)
        t_y = p1.tile([NP, 1], f32, tag="y")
        nc.vector.tensor_scalar(t_y[:].bitcast(i32), t_sh[:],
                                0x5F3759DF + 1, None, OP.add)
        t_xh = p1.tile([NP, 1], f32, tag="xh")
        nc.vector.tensor_scalar(t_xh[:], kn, -0.5, None, OP.mult)
        t_t1 = p1.tile([NP, 1], f32, tag="t1")
        t_t2 = p1.tile([NP, 1], f32, tag="t2")
        t_rsk = p1.tile([NP, 1], f32, tag="rsk")
        for it in range(2):
            nc.vector.tensor_tensor(t_t1[:], t_y[:], t_y[:], OP.mult)
            nc.vector.tensor_tensor(t_t2[:], t_t1[:], t_xh[:], OP.mult)
            nc.vector.tensor_scalar(t_t2[:], t_t2[:], 1.5, None, OP.add)
            dst = t_rsk[:] if it == 1 else t_y[:]
            nc.vector.tensor_tensor(dst, t_y[:], t_t2[:], OP.mult)
        # knh = sqrt(kn) = kn * rsqrt(kn)
        knh = tP[:, 4:5]
        nc.vector.tensor_tensor(knh, kn, t_rsk[:], OP.mult)
        kn2 = t_t1[:]
        nc.vector.tensor_tensor(kn2, kn, kn, OP.mult)
        kn4 = t_t2[:]
        nc.vector.tensor_tensor(kn4, kn2, kn2, OP.mult)
        # pow slots 5..11: kn^{2m}*knh for m=1..7
        nc.vector.tensor_tensor(tP[:, 5:6], kn2, knh, OP.mult)
        nc.vector.tensor_tensor(tP[:, 6:7], tP[:, 5:6], kn2, OP.mult)
        nc.vector.tensor_tensor(tP[:, 7:9], tP[:, 5:7],
                                kn4.broadcast_to((NP, 2)), OP.mult)
        kn8 = p1.tile([NP, 1], f32, tag="kn8")
        nc.vector.tensor_tensor(kn8[:], kn4, kn4, OP.mult)
        nc.vector.tensor_tensor(tP[:, 9:12], tP[:, 5:8],
                                kn8[:].broadcast_to((NP, 3)), OP.mult)
        nc.vector.tensor_tensor(tP[:, 32:33], kinv, tP[:, 3:4], OP.mult)
        nc.vector.scalar_tensor_tensor(tP[:, 64:65], kn, 0.5, enn,
                                       OP.mult, OP.mult)

        tPT = p1.tile([16, 128], f32r, tag="tPT")
        tKi = p1.tile([1, 128], f32r, tag="tKi")
        tWc = p1.tile([1, 128], f32, tag="tWc")
        tKn = p1.tile([NJ, 128], f32, tag="tKn")
        tW0 = p1.tile([1, FCH], f32r, tag="tW0")
        tJ0 = p1.tile([NJ, F], bf16, tag="J0")
        tCT = p1.tile([NJ, 2 * FCH], bf16, tag="CT")
        tCS = p1.tile([NJ, 2 * FCH], bf16, tag="CS")
        tMask = p1.tile([NJ, F], i32, tag="mask")
        tBgF = p1.tile([16, F], f32r, tag="bgF")
        tUV = p1.tile([NA, NP * 26], f32, tag="uv")
        tPL = p1.tile([NA, NZH * NP], f32r, tag="pl")
        tA1 = p1.tile([NA, NP], f32, tag="a1")
        tOut = p1.tile([NP, NZH * UP], f32, tag="out")

        c3 = tCS[:].rearrange("j (h p z) -> j p h z", h=2, z=NZH)
        uv3 = tUV[:].rearrange("q (pp c) -> q pp c", c=26)
        pl3 = tPL[:].rearrange("q (zz pp) -> q pp zz", pp=128)

        with ExitStack() as fld:
            p_t0 = fld.enter_context(
                tc.tile_pool(name="pt0", bufs=2, space="PSUM"))
            p_pj = fld.enter_context(
                tc.tile_pool(name="ppj", bufs=2, space="PSUM"))
            p_w = fld.enter_context(
                tc.tile_pool(name="pw", bufs=2, space="PSUM"))
            ppr = fld.enter_context(
                tc.tile_pool(name="ppr", bufs=2, space="PSUM"))

            # ---- one transpose: [128,65] -> [65,128] ----
            psP = p_t0.tile([65, 128], f32, tag="t0", name="psP")
            nc.tensor.transpose(psP[:], tP[:], tI[:])
            nc.vector.tensor_scalar(tPT[:], psP[0:16, :], 1.0, None, OP.mult)
            nc.vector.tensor_scalar(tKi[:], psP[32:33, :], 1.0, None, OP.mult)
            nc.vector.tensor_scalar(tWc[:], psP[64:65, :], 1.0, None, OP.mult)
            # kn replicated over 101 partitions (mask compare)
            psKn = p_t0.tile([NJ, 128], f32, tag="t0", name="psKn")
            nc.tensor.matmul(psKn[:], tLt[0:1, ON:ON + NJ], tPT[0:1, :],
                             start=True, stop=True)
            nc.vector.tensor_scalar(tKn[:], psKn[:], 1.0, None, OP.mult)
            # a1^2 = (S_AMP^2 Rinv)[a] * kinv[p]  (rank-1, PE outer product)
            psA1 = p_pj.tile([NA, 128], f32, tag="pj", name="psA1")
            nc.tensor.matmul(psA1[:], tLt[0:1, A1R:A1R + NA], tKi[0:1, :],
                             start=True, stop=True)
            nc.vector.tensor_scalar(tA1[:], psA1[:], 1.0, None, OP.mult)
            # c-stage w row: w0[(p,z)] = wc[p] * z
            w03 = tW0[:].rearrange("q (p z) -> q p z", z=NZH)
            nc.gpsimd.tensor_tensor(
                w03, tWc[:, :, None].broadcast_to((1, 128, NZH)),
                tLa[0:1, ZR:ZR + NZH][:, None, :].broadcast_to((1, 128, NZH)),
                OP.mult)

            def prep(c):
                if c >= 10:
                    return
                np_ = FP[c]
                p0 = FPO[c]
                sl = slice(p0 * NA, (p0 + np_) * NA)
                nc.gpsimd.tensor_tensor(
                    tBgF[:, sl].rearrange("k (p a) -> k p a", a=NA),
                    tPT[:, p0:p0 + np_][:, :, None]
                        .broadcast_to((16, np_, NA)),
                    tLa[:, AFT_:AFT_ + NA][:, None, :]
                        .broadcast_to((16, np_, NA)),
                    OP.mult)
                mvw = tMask[:, sl].rearrange("j (p a) -> j p a", a=NA)
                knb = tKn[:, p0:p0 + np_][:, :, None] \
                    .broadcast_to((NJ, np_, NA))
                thb = tTh[:, 0:NA][:, None, :].broadcast_to((NJ, np_, NA))
                nc.vector.tensor_tensor(mvw, knb.bitcast(i32),
                                        thb.bitcast(i32), OP.is_lt)

            def field_chunk(c):
                np_ = FP[c]
                p0 = FPO[c]
                w = np_ * NA
                sl = slice(p0 * NA, p0 * NA + w)
                rhs = tBgF[0:16, sl]
                ps_t0 = p_t0.tile([NJ, 512], f32, tag="t0", name="ps_t0")
                nc.tensor.matmul(ps_t0[:, 0:w], tLt[0:16, PH:PH + NJ], rhs,
                                 start=True, stop=True)
                prep(c + 1)
                # PSUM -> SBUF copy (Act even / DVE odd), then SBUF-side ops
                tT0 = pcos.tile([NJ, 512], f32, tag="t0s", name="tT0")
                if c % 2 == 0:
                    nc.scalar.activation(tT0[:, 0:w], ps_t0[:, 0:w], AF.Copy)
                else:
                    nc.vector.tensor_scalar(tT0[:, 0:w], ps_t0[:, 0:w], 1.0,
                                            None, OP.mult)
                tRR = pcos.tile([NJ, 512], f32, tag="rr", name="tRR")
                rre = (nc.gpsimd, nc.vector)[c % 2]
                rre.tensor_scalar(tRR[:, 0:w], tT0[:, 0:w], MAGIC,
                                  MAGIC, OP.add, OP.subtract)
                tU = pcos.tile([NJ, 512], f32, tag="u", name="tU")
                nc.gpsimd.tensor_tensor(tU[:, 0:w], tT0[:, 0:w],
                                        tRR[:, 0:w], OP.subtract)
                nc.scalar.activation(tJ0[:, sl], tU[:, 0:w], AF.Sin,
                                     scale=2.0 * PI)
                ps_pj = p_pj.tile([NJ, 512], f32, tag="pj", name="ps_pj")
                nc.tensor.matmul(ps_pj[:, 0:w], tLt[0:16, POL:POL + NJ], rhs,
                                 start=True, stop=True)
                nc.vector.copy_predicated(tJ0[:, sl], tMask[:, sl],
                                          ps_pj[:, 0:w])

            def c_chunk(k):
                np_ = CP[k]
                o = CPO[k] * NZH
                w = np_ * NZH
                ps_w = p_w.tile([NJ, 512], f32, tag="w", name="ps_w")
                nc.tensor.matmul(ps_w[:, 0:w], tLt[0:1, CL:CL + NJ],
                                 tW0[0:1, o:o + w], start=True, stop=True)
                tWs = pcos.tile([NJ, 512], f32, tag="ws", name="tWs")
                nc.vector.tensor_scalar(tWs[:, 0:w], ps_w[:, 0:w], 1.0,
                                        None, OP.mult)
                tRs = pcos.tile([NJ, 512], f32, tag="rr", name="tRs")
                rse = (nc.vector, nc.gpsimd)[k % 2]
                rse.tensor_scalar(tRs[:, 0:w], tWs[:, 0:w], MAGIC,
                                  MAGIC, OP.add, OP.subtract)
                tUs = pcos.tile([NJ, 512], f32, tag="u", name="tUs")
                nc.gpsimd.tensor_tensor(tUs[:, 0:w], tWs[:, 0:w],
                                        tRs[:, 0:w], OP.subtract)
                nc.scalar.activation(tCT[:, o:o + w], tUs[:, 0:w], AF.Sin,
                                     scale=2.0 * PI)
                cse = (nc.gpsimd, nc.vector)[k % 2]
                cse.tensor_scalar(tCS[:, o:o + w], tCT[:, o:o + w],
                                  tTh[:, NA:NA + 1], None, OP.mult)
                tXc = pcos.tile([NJ, 512], f32, tag="xc", name="tXc")
                nc.gpsimd.tensor_scalar(tXc[:, 0:w], tWs[:, 0:w], 0.25,
                                        None, OP.add)
                tRc = pcos.tile([NJ, 512], f32, tag="rr", name="tRc")
                rce = (nc.gpsimd, nc.vector)[k % 2]
                rce.tensor_scalar(tRc[:, 0:w], tXc[:, 0:w], MAGIC,
                                  MAGIC, OP.add, OP.subtract)
                tUc = pcos.tile([NJ, 512], f32, tag="u", name="tUc")
                nc.gpsimd.tensor_tensor(tUc[:, 0:w], tXc[:, 0:w],
                                        tRc[:, 0:w], OP.subtract)
                nc.scalar.activation(tCT[:, FCH + o:FCH + o + w],
                                     tUc[:, 0:w], AF.Sin, scale=2.0 * PI)
                nc.vector.tensor_scalar(tCS[:, FCH + o:FCH + o + w],
                                        tCT[:, FCH + o:FCH + o + w],
                                        tTh[:, NA:NA + 1], None, OP.mult)

            def wave(wv):
                tPRw = ppr.tile([NA, 512], f32, tag="PR", name="tPRw")
                for j in range(16):
                    p = wv * 16 + j
                    nc.tensor.matmul(tPRw[:, j * 32:j * 32 + 26],
                                     tJ0[:, p * NA:(p + 1) * NA],
                                     c3[:, p], start=True, stop=True)
                pr4 = tPRw[:].rearrange("q (n s) -> q n s", s=32)
                sl = slice(wv * 16, (wv + 1) * 16)
                nc.scalar.activation(uv3[:, sl, :], pr4[:, :, 0:26],
                                     AF.Square)
                nc.gpsimd.tensor_tensor(pl3[:, sl, :], uv3[:, sl, 0:NZH],
                                        uv3[:, sl, NZH:26], OP.add)
                nc.gpsimd.tensor_tensor(
                    pl3[:, sl, :], pl3[:, sl, :],
                    tA1[:, sl][:, :, None].broadcast_to((NA, 16, NZH)),
                    OP.mult)

            prep(0)
            for c in range(10):
                field_chunk(c)
                for k in C_AFTER.get(c, []):
                    c_chunk(k)
                for wv in WAVE_AFTER.get(c, []):
                    wave(wv)

        # ---- normalization + unique-radius expansion + output ----
        pnn = ctx.enter_context(tc.tile_pool(name="pnn", bufs=1, space="PSUM"))
        pgo = ctx.enter_context(tc.tile_pool(name="pgo", bufs=4, space="PSUM"))
        ps_n = pnn.tile([NP, 2], f32, tag="N")
        for zz in range(NZH):
            off = GU if zz == 0 else GU + 2
            nc.tensor.matmul(ps_n[:], tPL[:, zz * NP:(zz + 1) * NP],
                             tGt[:, off:off + 2],
                             start=(zz == 0), stop=(zz == NZH - 1))
        t_rcp = p1.tile([NP, 1], f32, tag="rcp")
        nc.vector.reciprocal(t_rcp[:], ps_n[:, 0:1])
        GZ = [(0, 4), (4, 4), (8, 4), (12, 1)]
        for g, (z0, nz) in enumerate(GZ):
            tOC = pgo.tile([NP, 4 * UP], f32, tag="OC", name="tOC")
            for k in range(nz):
                nc.tensor.matmul(tOC[:, k * UP:(k + 1) * UP],
                                 tPL[:, (z0 + k) * NP:(z0 + k + 1) * NP],
                                 tGt[:, 0:UP], start=True, stop=True)
            osl = tOut[:, z0 * UP:(z0 + nz) * UP]
            if g % 2 == 1:
                nc.scalar.activation(osl, tOC[:, 0:nz * UP], AF.Copy,
                                     scale=t_rcp[:, 0:1])
            else:
                nc.vector.tensor_scalar(osl, tOC[:, 0:nz * UP],
                                        t_rcp[:, 0:1], None, OP.mult)
            deng = (nc.sync, nc.gpsimd, nc.sync, nc.gpsimd)[g]
            deng.dma_start(out=d_out[:, z0 * UP:(z0 + nz) * UP],
                           in_=tOut[:, z0 * UP:(z0 + nz) * UP])

    nc.finalize()
    _CACHE["nc"] = nc
    return nc


def _expand(core_out):
    """[NP, 13*U] device output -> [NP, 25, 25, 25] full psf (pure gather)."""
    r = core_out.reshape(NP, NZH, UP)
    p625 = r[:, :, UIDX]
    full = np.concatenate([p625[:, 12:0:-1, :], p625], axis=1)
    return full.reshape(NP, NZ, 25, 25)


def kernel(params):
    _ensure_paths()
    from concourse.bass_utils import run_bass_kernel_spmd

    params = np.asarray(params, dtype=np.float32)
    assert params.shape == (B, CH, 2)
    consts = _host_consts()
    nc = _build_nc()

    per = B // NCORES
    in_maps = []
    for i in range(NCORES):
        m = {"params": params[i * per:(i + 1) * per].reshape(NP, 2).copy()}
        m.update(consts)
        in_maps.append(m)

    res = run_bass_kernel_spmd(nc, in_maps, list(range(NCORES)))

    out = np.empty((B, CH, NZ, 25, 25), np.float32)
    for i in range(NCORES):
        full = _expand(np.asarray(res.results[i]["out"]))
        out[i * per:(i + 1) * per] = full.reshape(per, CH, NZ, 25, 25)
    return out
